# revision 13
# baseline (speedup 1.0000x reference)
"""TRN2 Bass kernel for nn_BiDecoder — M-table design.

ratings[e] = sum_r softmax_r(ufeat[src[e]] @ Ps[r] @ ifeat[dst[e]]) * (r+1)

Sharding: edges sorted by dst -> 8 contiguous shards (core item band <= 8192
rows). Inside a core, edges are bucketed into cells (src-quarter q, item-block
k); each cell gets a static tile quota (max over cores) so one program serves
all cores. Per 128-edge tile (all edges in one quarter and one 128-item
block):
  - m_tile[e, (r,f)] = M[dst[e], (r,f)] is expanded from the on-device table
    M = ifeat_band @ PsT (f32r) by a staircase selection matmul: Sel[j, e] =
    (s_j <= e < e_j) built from two DVE is_ge compares on an iota row.
  - us rows arrive via gpsimd dma_gather (int16 idx into one of four static
    ufeat windows) — the only per-edge descriptor stream, which bounds the
    kernel (~8.7us of Q7 descriptor generation per 1024 rows).
  - DVE: b = m * us (f16 out), two-stage f16 reduce -> scores; softmax
    batched over two gather groups.
"""
import sys

sys.path.insert(0, "/opt/trn_rl_repo")
import numpy as np

P = 128
D = 64
R = 5
RD = R * D
N_USERS, N_ITEMS, E = 100000, 50000, 1000000
N_CORES = 8
E_CORE = E // N_CORES
IF_ROWS = 8192
KBLKS = IF_ROWS // P  # 64 item-blocks per band
NQUART = 4  # src classes (three 32768-row windows + tail)
CLS_BASE = [0, 32768, 65536, 98304]
CLS_SIZE = [32768, 32768, 32768, N_USERS - 98304]
GBLK = 1024  # idx per dma_gather call (ucode limit)
TPG = GBLK // P  # tiles per gather group = 8

_NC_CACHE = {}


def _layout(src, dst):
    """Static-per-program layout: per-cell tile quotas (max over cores), the
    tile list, and per-core slot fills."""
    perm = np.argsort(dst, kind="stable")
    cores = []
    quota = np.zeros((NQUART, KBLKS), np.int64)
    for c in range(N_CORES):
        eids = perm[c * E_CORE : (c + 1) * E_CORE]
        s = src[eids].astype(np.int64)
        d = dst[eids].astype(np.int64)
        d_lo = int(d.min())
        width = int(d.max()) - d_lo + 1
        assert width <= IF_ROWS, width
        loc = d - d_lo
        q = np.minimum(s // 32768, 3)
        k = loc // P
        cnt = np.bincount(q * KBLKS + k, minlength=NQUART * KBLKS).reshape(
            NQUART, KBLKS
        )
        quota = np.maximum(quota, (cnt + P - 1) // P)
        cores.append((eids, s, loc, q, k, d_lo))

    tiles = []  # (q, k) per tile; quarter-major, padded to TPG per quarter
    group_quarter = []
    for qq in range(NQUART):
        start = len(tiles)
        for kk in range(KBLKS):
            tiles.extend([(qq, kk)] * int(quota[qq, kk]))
        while (len(tiles) - start) % TPG != 0:
            tiles.append((qq, 0))
        group_quarter.extend([qq] * ((len(tiles) - start) // TPG))
    NT = len(tiles)
    NG = NT // TPG
    return perm, cores, quota, tiles, group_quarter, NT, NG


def _build_kernel(layout_sig, tiles, group_quarter, NT, NG):
    import concourse.bacc as bacc
    import concourse.mybir as mybir
    import concourse.tile as tile
    from concourse import library_config

    nc = bacc.Bacc(None, target_bir_lowering=False)
    f32, i16, bf16 = mybir.dt.float32, mybir.dt.int16, mybir.dt.bfloat16
    f16 = mybir.dt.float16
    f32r = mybir.dt.float32r
    assert NG % 2 == 0, NG

    ufeat_d = nc.dram_tensor("ufeat", [N_USERS, D], f32, kind="ExternalInput")
    ifT_d = nc.dram_tensor("ifT", [D, IF_ROWS], f32r, kind="ExternalInput")
    psT_d = nc.dram_tensor("psT", [D, RD], f32r, kind="ExternalInput")
    iota_d = nc.dram_tensor("iota", [P, P], f16, kind="ExternalInput")
    srow_d = nc.dram_tensor("srow", [P, NT], f16, kind="ExternalInput")
    snext_d = nc.dram_tensor("snext", [P, NT], f16, kind="ExternalInput")
    idxu_d = nc.dram_tensor("idxu", [P, NG * (GBLK // 16)], i16, kind="ExternalInput")
    vals_d = nc.dram_tensor("vals", [P, R], f32, kind="ExternalInput")
    out_d = nc.dram_tensor("out", [P, NG * TPG], f32, kind="ExternalOutput")

    with tile.TileContext(nc) as tc:
        nc.gpsimd.load_library(library_config.mlp)
        with (
            tc.tile_pool(name="const", bufs=1) as cpool,
            tc.tile_pool(name="gatheru", bufs=4) as gupool,
            tc.tile_pool(name="work", bufs=2) as wpool,
            tc.tile_pool(name="psum_m", bufs=4, space="PSUM") as mpool,
            tc.tile_pool(name="psum_b", bufs=2, space="PSUM") as bpool,
        ):
            ifT = cpool.tile([D, IF_ROWS], f32r)
            nc.sync.dma_start(ifT[:], ifT_d[:])
            psT = cpool.tile([D, RD], f32r)
            nc.sync.dma_start(psT[:], psT_d[:])
            iota_t = cpool.tile([P, P], f16)
            nc.sync.dma_start(iota_t[:], iota_d[:])
            srow = cpool.tile([P, NT], f16)
            nc.sync.dma_start(srow[:], srow_d[:])
            snext = cpool.tile([P, NT], f16)
            nc.sync.dma_start(snext[:], snext_d[:])
            idxu = cpool.tile([P, NG * (GBLK // 16)], i16)
            nc.sync.dma_start(idxu[:, 0 : 4 * (GBLK // 16)], idxu_d[:, 0 : 4 * (GBLK // 16)])
            nc.sync.dma_start(
                idxu[:, 4 * (GBLK // 16) :], idxu_d[:, 4 * (GBLK // 16) :]
            )
            vals_t = cpool.tile([P, R], f32)
            nc.sync.dma_start(vals_t[:], vals_d[:])

            # M[j, (r,f)] = sum_g ifeat[j, g] * psT[g, (r,f)], stored f16
            M_sb = cpool.tile([P, KBLKS * RD], f16)
            for k in range(KBLKS):
                M_ps = bpool.tile([P, RD], f32, tag="Mps")
                nc.tensor.matmul(
                    M_ps[:], lhsT=ifT[:, k * P : (k + 1) * P], rhs=psT[:]
                )
                nc.scalar.copy(M_sb[:, k * RD : (k + 1) * RD], M_ps[:])

            scores2 = None
            for g in range(NG):
                gq = group_quarter[g]
                us_g = gupool.tile([P, TPG * D], f32, tag="us")
                s0 = g * (GBLK // 16)
                nc.gpsimd.dma_gather(
                    out_ap=us_g[:].rearrange("p (t d) -> p t d", d=D),
                    in_ap=ufeat_d[CLS_BASE[gq] : CLS_BASE[gq] + CLS_SIZE[gq], :],
                    idxs_ap=idxu[:, s0 : s0 + GBLK // 16],
                    num_idxs=GBLK,
                    num_idxs_reg=GBLK,
                    elem_size=D,
                    single_packet=False,
                )

                # staircase selectors for the whole group (8 tiles)
                selA = wpool.tile([P, TPG * P], f16, tag="selA")
                nc.vector.tensor_tensor(
                    out=selA[:].rearrange("p (t e) -> p t e", t=TPG),
                    in0=iota_t[:]
                    .rearrange("p (o e) -> p o e", o=1)
                    .to_broadcast([P, TPG, P]),
                    in1=srow[:, g * TPG : (g + 1) * TPG]
                    .rearrange("p (t o) -> p t o", o=1)
                    .to_broadcast([P, TPG, P]),
                    op=mybir.AluOpType.is_ge,
                )
                selB = wpool.tile([P, TPG * P], f16, tag="selB")
                nc.vector.tensor_tensor(
                    out=selB[:].rearrange("p (t e) -> p t e", t=TPG),
                    in0=iota_t[:]
                    .rearrange("p (o e) -> p o e", o=1)
                    .to_broadcast([P, TPG, P]),
                    in1=snext[:, g * TPG : (g + 1) * TPG]
                    .rearrange("p (t o) -> p t o", o=1)
                    .to_broadcast([P, TPG, P]),
                    op=mybir.AluOpType.is_ge,
                )
                selD = wpool.tile([P, TPG * P], f16, tag="selD")
                nc.vector.tensor_tensor(
                    out=selD[:], in0=selA[:], in1=selB[:],
                    op=mybir.AluOpType.subtract,
                )

                m_big = wpool.tile([P, TPG * RD], f16, tag="mbig")
                for ti in range(TPG):
                    t = g * TPG + ti
                    k = tiles[t][1]
                    m_ps = mpool.tile([P, RD], f32, tag="m")
                    nc.tensor.matmul(
                        m_ps[:],
                        lhsT=selD[:, ti * P : (ti + 1) * P],
                        rhs=M_sb[:, k * RD : (k + 1) * RD],
                    )
                    nc.scalar.copy(m_big[:, ti * RD : (ti + 1) * RD], m_ps[:])

                b_big = wpool.tile([P, TPG * RD], f16, tag="bbig")
                us_bc = (
                    us_g[:]
                    .rearrange("p (t o d) -> p t o d", t=TPG, o=1)
                    .to_broadcast([P, TPG, R, D])
                )
                nc.vector.tensor_mul(
                    b_big[:].rearrange("p (t r d) -> p t r d", t=TPG, r=R),
                    m_big[:].rearrange("p (t r d) -> p t r d", t=TPG, r=R),
                    us_bc,
                )
                b1 = wpool.tile([P, TPG * R * 8], f16, tag="b1")
                with nc.allow_low_precision(reason="f16 partials validated on host"):
                    nc.vector.tensor_reduce(
                        out=b1[:],
                        in_=b_big[:].rearrange(
                            "p (q x) -> p q x", x=8
                        ),
                        axis=mybir.AxisListType.X,
                        op=mybir.AluOpType.add,
                    )
                if g % 2 == 0:
                    scores2 = wpool.tile([P, 2 * TPG * R], f32, tag="scores2")
                nc.vector.tensor_reduce(
                    out=scores2[:, (g % 2) * TPG * R : ((g % 2) + 1) * TPG * R],
                    in_=b1[:].rearrange("p (q a) -> p q a", a=8),
                    axis=mybir.AxisListType.X,
                    op=mybir.AluOpType.add,
                )
                if g % 2 == 1:
                    NTT = 2 * TPG
                    e_t = wpool.tile([P, NTT * R], f32, tag="e")
                    nc.scalar.activation(
                        e_t[:], scores2[:], mybir.ActivationFunctionType.Exp
                    )
                    den = wpool.tile([P, NTT], f32, tag="den")
                    nc.vector.tensor_reduce(
                        out=den[:],
                        in_=e_t[:].rearrange("p (t r) -> p t r", r=R),
                        axis=mybir.AxisListType.X,
                        op=mybir.AluOpType.add,
                    )
                    num_s = wpool.tile([P, NTT * R], f32, tag="nums")
                    vals_bc = (
                        vals_t[:]
                        .rearrange("p (o r) -> p o r", o=1)
                        .to_broadcast([P, NTT, R])
                    )
                    nc.vector.tensor_mul(
                        num_s[:].rearrange("p (t r) -> p t r", r=R),
                        e_t[:].rearrange("p (t r) -> p t r", r=R),
                        vals_bc,
                    )
                    num = wpool.tile([P, NTT], f32, tag="num")
                    nc.vector.tensor_reduce(
                        out=num[:],
                        in_=num_s[:].rearrange("p (t r) -> p t r", r=R),
                        axis=mybir.AxisListType.X,
                        op=mybir.AluOpType.add,
                    )
                    rden = wpool.tile([P, NTT], f32, tag="rden")
                    nc.vector.reciprocal(rden[:], den[:])
                    rat = wpool.tile([P, NTT], f32, tag="rat")
                    nc.vector.tensor_mul(rat[:], num[:], rden[:])
                    nc.sync.dma_start(
                        out_d[:, (g - 1) * TPG : (g + 1) * TPG], rat[:]
                    )
    nc.compile()
    return nc


def _prepare(ufeat, ifeat, Ps, src, dst):
    src = np.asarray(src, np.int64)
    dst = np.asarray(dst, np.int64)
    perm, cores, quota, tiles, group_quarter, NT, NG = _layout(src, dst)
    _NC_CACHE["layout"] = (tiles, group_quarter, NT, NG)

    psT = np.ascontiguousarray(
        np.asarray(Ps, np.float32).transpose(2, 0, 1).reshape(D, RD)
    )
    iota = np.tile(np.arange(P, dtype=np.float32), (P, 1))
    vals = np.tile(np.arange(1.0, 6.0, dtype=np.float32), (P, 1)).astype(np.float32)

    tiles_arr = np.asarray(tiles, np.int64)  # [NT, 2]
    in_maps, metas = [], []
    for c in range(N_CORES):
        eids, s, loc, q, k, d_lo = cores[c]
        ifT = np.zeros((D, IF_ROWS), np.float32)
        width = int(loc.max()) + 1
        ifT[:, :width] = ifeat[d_lo : d_lo + width].T

        # bucket edges per cell, preserving dst order
        cell_of = q * KBLKS + k
        order = np.argsort(cell_of, kind="stable")  # edges grouped by cell
        cell_sorted = cell_of[order]
        bounds = np.searchsorted(cell_sorted, np.arange(NQUART * KBLKS + 1))

        idxu16 = np.zeros(NT * P, np.int16)
        srow = np.zeros((P, NT), np.float32)
        slot_map = np.full(NT * P, -1, np.int64)

        # per-cell cursor into its tile quota
        cell_tile_start = {}
        for t, (qq, kk) in enumerate(tiles):
            cell = qq * KBLKS + kk
            cell_tile_start.setdefault(cell, []).append(t)
        for cell, tlist in cell_tile_start.items():
            lo, hi = bounds[cell], bounds[cell + 1]
            ed = order[lo:hi]  # edge positions in eids order (dst-sorted)
            n = hi - lo
            for i, t in enumerate(tlist):
                seg = ed[i * P : (i + 1) * P]
                m = len(seg)
                base = t * P
                if m > 0:
                    rows = loc[seg] - (cell % KBLKS) * P  # 0..127
                    idxu16[base : base + m] = (
                        s[seg] - CLS_BASE[cell // KBLKS]
                    ).astype(np.int16)
                    slot_map[base : base + m] = eids[seg]
                    # staircase: rows sorted ascending (dst order)
                    rr = rows.astype(np.int64)
                    starts = np.searchsorted(rr, np.arange(P), side="left")
                    srow[:, t] = starts
                # else: fully dummy tile: srow stays 0 (positions land on
                # row 127 via snext[127]=128); idxu stays 0

        def wrap(a):
            cols = np.concatenate(
                [
                    a[gg * GBLK : (gg + 1) * GBLK].reshape(GBLK // 16, 16).T
                    for gg in range(len(a) // GBLK)
                ],
                axis=1,
            )
            return np.tile(cols, (8, 1)).astype(np.int16)

        in_maps.append(
            {
                "ufeat": np.ascontiguousarray(ufeat, np.float32),
                "ifT": np.ascontiguousarray(ifT),
                "psT": psT,
                "iota": iota.astype(np.float16),
                "srow": srow.astype(np.float16),
                "snext": np.vstack([srow[1:], np.full((1, srow.shape[1]), 128.0, np.float32)]).astype(np.float16),
                "idxu": wrap(idxu16),
                "vals": vals,
            }
        )
        metas.append(slot_map)

    return in_maps, metas


def _install_profile_hook():
    import types

    try:
        from antenv.axon_hooks import get_axon_ntff_profile_hook  # noqa: F401

        return
    except ImportError:
        pass
    import antenv
    from trn_agent_boot.trn_boot import _ntff_profile_via_ctypes

    hook = _ntff_profile_via_ctypes("/opt/axon/libaxon_pjrt.so")
    mod = types.ModuleType("antenv.axon_hooks")
    mod._hook = hook
    mod.get_axon_ntff_profile_hook = lambda: mod._hook
    mod.set_axon_ntff_profile_hook = lambda h: setattr(mod, "_hook", h)
    sys.modules["antenv.axon_hooks"] = mod
    antenv.axon_hooks = mod


def kernel(ufeat, ifeat, Ps, src, dst):
    from concourse.bass_utils import run_bass_kernel_spmd

    ufeat = np.asarray(ufeat, np.float32)
    ifeat = np.asarray(ifeat, np.float32)
    Ps = np.asarray(Ps, np.float32)
    src = np.asarray(src, np.int32)
    dst = np.asarray(dst, np.int32)

    in_maps, metas = _prepare(ufeat, ifeat, Ps, src, dst)
    tiles, group_quarter, NT, NG = _NC_CACHE["layout"]
    if "nc" not in _NC_CACHE:
        _NC_CACHE["nc"] = _build_kernel(None, tiles, group_quarter, NT, NG)
    nc = _NC_CACHE["nc"]
    res = run_bass_kernel_spmd(nc, in_maps, core_ids=list(range(N_CORES)))
    out = np.zeros(E, np.float32)
    for c in range(N_CORES):
        o = res.results[c]["out"]  # [P, NG*TPG]
        flat = o.T.reshape(-1)  # slot id = t*128 + p
        sm = metas[c]
        valid = sm >= 0
        out[sm[valid]] = flat[valid]
    return out


# revision 14
# speedup vs baseline: 1.1920x; 1.1920x over previous
"""TRN2 Bass kernel for nn_BiDecoder — M-table design.

ratings[e] = sum_r softmax_r(ufeat[src[e]] @ Ps[r] @ ifeat[dst[e]]) * (r+1)

Sharding: edges sorted by dst -> 8 contiguous shards (core item band <= 8192
rows). Inside a core, edges are bucketed into cells (src-quarter q, item-block
k); each cell gets a static tile quota (max over cores) so one program serves
all cores. Per 128-edge tile (all edges in one quarter and one 128-item
block):
  - m_tile[e, (r,f)] = M[dst[e], (r,f)] is expanded from the on-device table
    M = ifeat_band @ PsT (f32r) by a staircase selection matmul: Sel[j, e] =
    (s_j <= e < e_j) built from two DVE is_ge compares on an iota row.
  - us rows arrive via gpsimd dma_gather (int16 idx into one of four static
    ufeat windows) — the only per-edge descriptor stream, which bounds the
    kernel (~8.7us of Q7 descriptor generation per 1024 rows).
  - DVE: b = m * us (f16 out), two-stage f16 reduce -> scores; softmax
    batched over two gather groups.
"""
import sys

sys.path.insert(0, "/opt/trn_rl_repo")
import numpy as np

P = 128
D = 64
R = 5
RD = R * D
N_USERS, N_ITEMS, E = 100000, 50000, 1000000
N_CORES = 8
E_CORE = E // N_CORES
IF_ROWS = 8192
KBLKS = IF_ROWS // P  # 64 item-blocks per band
NQUART = 4  # src classes (three 32768-row windows + tail)
CLS_BASE = [0, 32768, 65536, 98304]
CLS_SIZE = [32768, 32768, 32768, N_USERS - 98304]
GBLK = 1024  # idx per dma_gather call (ucode limit)
TPG = GBLK // P  # tiles per gather group = 8

_NC_CACHE = {}


def _layout(src, dst):
    """Static-per-program layout: per-cell tile quotas (max over cores), the
    tile list, and per-core slot fills."""
    perm = np.argsort(dst, kind="stable")
    cores = []
    quota = np.zeros((NQUART, KBLKS), np.int64)
    for c in range(N_CORES):
        eids = perm[c * E_CORE : (c + 1) * E_CORE]
        s = src[eids].astype(np.int64)
        d = dst[eids].astype(np.int64)
        d_lo = int(d.min())
        width = int(d.max()) - d_lo + 1
        assert width <= IF_ROWS, width
        loc = d - d_lo
        q = np.minimum(s // 32768, 3)
        k = loc // P
        cnt = np.bincount(q * KBLKS + k, minlength=NQUART * KBLKS).reshape(
            NQUART, KBLKS
        )
        quota = np.maximum(quota, (cnt + P - 1) // P)
        cores.append((eids, s, loc, q, k, d_lo))

    tiles = []  # (q, k) per tile; quarter-major, padded to TPG per quarter
    group_quarter = []
    for qq in range(NQUART):
        start = len(tiles)
        for kk in range(KBLKS):
            tiles.extend([(qq, kk)] * int(quota[qq, kk]))
        while (len(tiles) - start) % TPG != 0:
            tiles.append((qq, 0))
        group_quarter.extend([qq] * ((len(tiles) - start) // TPG))
    NT = len(tiles)
    NG = NT // TPG
    return perm, cores, quota, tiles, group_quarter, NT, NG


def _build_kernel(layout_sig, tiles, group_quarter, NT, NG):
    import concourse.bacc as bacc
    import concourse.mybir as mybir
    import concourse.tile as tile
    from concourse import library_config

    nc = bacc.Bacc(None, target_bir_lowering=False)
    f32, i16, bf16 = mybir.dt.float32, mybir.dt.int16, mybir.dt.bfloat16
    f16 = mybir.dt.float16
    f32r = mybir.dt.float32r
    assert NG % 2 == 0, NG

    ufeat_d = nc.dram_tensor("ufeat", [N_USERS, D], f32, kind="ExternalInput")
    ifT_d = nc.dram_tensor("ifT", [D, IF_ROWS], f32r, kind="ExternalInput")
    psT_d = nc.dram_tensor("psT", [D, RD], f32r, kind="ExternalInput")
    iota_d = nc.dram_tensor("iota", [P, P], f16, kind="ExternalInput")
    srow_d = nc.dram_tensor("srow", [P, NT], f16, kind="ExternalInput")
    snext_d = nc.dram_tensor("snext", [P, NT], f16, kind="ExternalInput")
    idxu_d = nc.dram_tensor("idxu", [P, NG * (GBLK // 16)], i16, kind="ExternalInput")
    vals_d = nc.dram_tensor("vals", [P, R], f32, kind="ExternalInput")
    out_d = nc.dram_tensor("out", [P, NG * TPG], f32, kind="ExternalOutput")

    with tile.TileContext(nc) as tc:
        nc.gpsimd.load_library(library_config.mlp)
        with (
            tc.tile_pool(name="const", bufs=1) as cpool,
            tc.tile_pool(name="gatheru", bufs=4) as gupool,
            tc.tile_pool(name="work", bufs=2) as wpool,
            tc.tile_pool(name="psum_m", bufs=4, space="PSUM") as mpool,
            tc.tile_pool(name="psum_b", bufs=2, space="PSUM") as bpool,
        ):
            ifT = cpool.tile([D, IF_ROWS], f32r)
            nc.sync.dma_start(ifT[:], ifT_d[:])
            psT = cpool.tile([D, RD], f32r)
            nc.sync.dma_start(psT[:], psT_d[:])
            iota_t = cpool.tile([P, P], f16)
            nc.sync.dma_start(iota_t[:], iota_d[:])
            srow = cpool.tile([P, NT], f16)
            nc.sync.dma_start(srow[:], srow_d[:])
            snext = cpool.tile([P, NT], f16)
            nc.sync.dma_start(snext[:], snext_d[:])
            idxu = cpool.tile([P, NG * (GBLK // 16)], i16)
            nc.sync.dma_start(idxu[:, 0 : 4 * (GBLK // 16)], idxu_d[:, 0 : 4 * (GBLK // 16)])
            nc.sync.dma_start(
                idxu[:, 4 * (GBLK // 16) :], idxu_d[:, 4 * (GBLK // 16) :]
            )
            vals_t = cpool.tile([P, R], f32)
            nc.sync.dma_start(vals_t[:], vals_d[:])

            # M[j, (r,f)] = sum_g ifeat[j, g] * psT[g, (r,f)], stored f16
            M_sb = cpool.tile([P, KBLKS * RD], f16)
            for k in range(KBLKS):
                M_ps = bpool.tile([P, RD], f32, tag="Mps")
                nc.tensor.matmul(
                    M_ps[:], lhsT=ifT[:, k * P : (k + 1) * P], rhs=psT[:]
                )
                nc.scalar.copy(M_sb[:, k * RD : (k + 1) * RD], M_ps[:])

            scores2 = None
            for g in range(NG):
                gq = group_quarter[g]
                us_g = gupool.tile([P, TPG * D], f32, tag="us")
                s0 = g * (GBLK // 16)
                nc.gpsimd.dma_gather(
                    out_ap=us_g[:].rearrange("p (t d) -> p t d", d=D),
                    in_ap=ufeat_d[CLS_BASE[gq] : CLS_BASE[gq] + CLS_SIZE[gq], :],
                    idxs_ap=idxu[:, s0 : s0 + GBLK // 16],
                    num_idxs=GBLK,
                    num_idxs_reg=GBLK,
                    elem_size=D,
                )

                # staircase selectors for the whole group (8 tiles)
                selA = wpool.tile([P, TPG * P], f16, tag="selA")
                nc.vector.tensor_tensor(
                    out=selA[:].rearrange("p (t e) -> p t e", t=TPG),
                    in0=iota_t[:]
                    .rearrange("p (o e) -> p o e", o=1)
                    .to_broadcast([P, TPG, P]),
                    in1=srow[:, g * TPG : (g + 1) * TPG]
                    .rearrange("p (t o) -> p t o", o=1)
                    .to_broadcast([P, TPG, P]),
                    op=mybir.AluOpType.is_ge,
                )
                selB = wpool.tile([P, TPG * P], f16, tag="selB")
                nc.vector.tensor_tensor(
                    out=selB[:].rearrange("p (t e) -> p t e", t=TPG),
                    in0=iota_t[:]
                    .rearrange("p (o e) -> p o e", o=1)
                    .to_broadcast([P, TPG, P]),
                    in1=snext[:, g * TPG : (g + 1) * TPG]
                    .rearrange("p (t o) -> p t o", o=1)
                    .to_broadcast([P, TPG, P]),
                    op=mybir.AluOpType.is_ge,
                )
                selD = wpool.tile([P, TPG * P], f16, tag="selD")
                nc.vector.tensor_tensor(
                    out=selD[:], in0=selA[:], in1=selB[:],
                    op=mybir.AluOpType.subtract,
                )

                m_big = wpool.tile([P, TPG * RD], f16, tag="mbig")
                for ti in range(TPG):
                    t = g * TPG + ti
                    k = tiles[t][1]
                    m_ps = mpool.tile([P, RD], f32, tag="m")
                    nc.tensor.matmul(
                        m_ps[:],
                        lhsT=selD[:, ti * P : (ti + 1) * P],
                        rhs=M_sb[:, k * RD : (k + 1) * RD],
                    )
                    nc.scalar.copy(m_big[:, ti * RD : (ti + 1) * RD], m_ps[:])

                b_big = wpool.tile([P, TPG * RD], f16, tag="bbig")
                us_bc = (
                    us_g[:]
                    .rearrange("p (t o d) -> p t o d", t=TPG, o=1)
                    .to_broadcast([P, TPG, R, D])
                )
                nc.vector.tensor_mul(
                    b_big[:].rearrange("p (t r d) -> p t r d", t=TPG, r=R),
                    m_big[:].rearrange("p (t r d) -> p t r d", t=TPG, r=R),
                    us_bc,
                )
                b1 = wpool.tile([P, TPG * R * 8], f16, tag="b1")
                with nc.allow_low_precision(reason="f16 partials validated on host"):
                    nc.vector.tensor_reduce(
                        out=b1[:],
                        in_=b_big[:].rearrange(
                            "p (q x) -> p q x", x=8
                        ),
                        axis=mybir.AxisListType.X,
                        op=mybir.AluOpType.add,
                    )
                if g % 2 == 0:
                    scores2 = wpool.tile([P, 2 * TPG * R], f32, tag="scores2")
                nc.vector.tensor_reduce(
                    out=scores2[:, (g % 2) * TPG * R : ((g % 2) + 1) * TPG * R],
                    in_=b1[:].rearrange("p (q a) -> p q a", a=8),
                    axis=mybir.AxisListType.X,
                    op=mybir.AluOpType.add,
                )
                if g % 2 == 1:
                    NTT = 2 * TPG
                    e_t = wpool.tile([P, NTT * R], f32, tag="e")
                    nc.scalar.activation(
                        e_t[:], scores2[:], mybir.ActivationFunctionType.Exp
                    )
                    den = wpool.tile([P, NTT], f32, tag="den")
                    nc.vector.tensor_reduce(
                        out=den[:],
                        in_=e_t[:].rearrange("p (t r) -> p t r", r=R),
                        axis=mybir.AxisListType.X,
                        op=mybir.AluOpType.add,
                    )
                    num_s = wpool.tile([P, NTT * R], f32, tag="nums")
                    vals_bc = (
                        vals_t[:]
                        .rearrange("p (o r) -> p o r", o=1)
                        .to_broadcast([P, NTT, R])
                    )
                    nc.vector.tensor_mul(
                        num_s[:].rearrange("p (t r) -> p t r", r=R),
                        e_t[:].rearrange("p (t r) -> p t r", r=R),
                        vals_bc,
                    )
                    num = wpool.tile([P, NTT], f32, tag="num")
                    nc.vector.tensor_reduce(
                        out=num[:],
                        in_=num_s[:].rearrange("p (t r) -> p t r", r=R),
                        axis=mybir.AxisListType.X,
                        op=mybir.AluOpType.add,
                    )
                    rden = wpool.tile([P, NTT], f32, tag="rden")
                    nc.vector.reciprocal(rden[:], den[:])
                    rat = wpool.tile([P, NTT], f32, tag="rat")
                    nc.vector.tensor_mul(rat[:], num[:], rden[:])
                    nc.sync.dma_start(
                        out_d[:, (g - 1) * TPG : (g + 1) * TPG], rat[:]
                    )
    nc.compile()
    return nc


def _prepare(ufeat, ifeat, Ps, src, dst):
    src = np.asarray(src, np.int64)
    dst = np.asarray(dst, np.int64)
    perm, cores, quota, tiles, group_quarter, NT, NG = _layout(src, dst)
    _NC_CACHE["layout"] = (tiles, group_quarter, NT, NG)

    psT = np.ascontiguousarray(
        np.asarray(Ps, np.float32).transpose(2, 0, 1).reshape(D, RD)
    )
    iota = np.tile(np.arange(P, dtype=np.float32), (P, 1))
    vals = np.tile(np.arange(1.0, 6.0, dtype=np.float32), (P, 1)).astype(np.float32)

    tiles_arr = np.asarray(tiles, np.int64)  # [NT, 2]
    in_maps, metas = [], []
    for c in range(N_CORES):
        eids, s, loc, q, k, d_lo = cores[c]
        ifT = np.zeros((D, IF_ROWS), np.float32)
        width = int(loc.max()) + 1
        ifT[:, :width] = ifeat[d_lo : d_lo + width].T

        # bucket edges per cell, preserving dst order
        cell_of = q * KBLKS + k
        order = np.argsort(cell_of, kind="stable")  # edges grouped by cell
        cell_sorted = cell_of[order]
        bounds = np.searchsorted(cell_sorted, np.arange(NQUART * KBLKS + 1))

        idxu16 = np.zeros(NT * P, np.int16)
        srow = np.zeros((P, NT), np.float32)
        slot_map = np.full(NT * P, -1, np.int64)

        # per-cell cursor into its tile quota
        cell_tile_start = {}
        for t, (qq, kk) in enumerate(tiles):
            cell = qq * KBLKS + kk
            cell_tile_start.setdefault(cell, []).append(t)
        for cell, tlist in cell_tile_start.items():
            lo, hi = bounds[cell], bounds[cell + 1]
            ed = order[lo:hi]  # edge positions in eids order (dst-sorted)
            n = hi - lo
            for i, t in enumerate(tlist):
                seg = ed[i * P : (i + 1) * P]
                m = len(seg)
                base = t * P
                if m > 0:
                    rows = loc[seg] - (cell % KBLKS) * P  # 0..127
                    idxu16[base : base + m] = (
                        s[seg] - CLS_BASE[cell // KBLKS]
                    ).astype(np.int16)
                    slot_map[base : base + m] = eids[seg]
                    # staircase: rows sorted ascending (dst order)
                    rr = rows.astype(np.int64)
                    starts = np.searchsorted(rr, np.arange(P), side="left")
                    srow[:, t] = starts
                # else: fully dummy tile: srow stays 0 (positions land on
                # row 127 via snext[127]=128); idxu stays 0

        def wrap(a):
            cols = np.concatenate(
                [
                    a[gg * GBLK : (gg + 1) * GBLK].reshape(GBLK // 16, 16).T
                    for gg in range(len(a) // GBLK)
                ],
                axis=1,
            )
            return np.tile(cols, (8, 1)).astype(np.int16)

        in_maps.append(
            {
                "ufeat": np.ascontiguousarray(ufeat, np.float32),
                "ifT": np.ascontiguousarray(ifT),
                "psT": psT,
                "iota": iota.astype(np.float16),
                "srow": srow.astype(np.float16),
                "snext": np.vstack([srow[1:], np.full((1, srow.shape[1]), 128.0, np.float32)]).astype(np.float16),
                "idxu": wrap(idxu16),
                "vals": vals,
            }
        )
        metas.append(slot_map)

    return in_maps, metas


def _install_profile_hook():
    import types

    try:
        from antenv.axon_hooks import get_axon_ntff_profile_hook  # noqa: F401

        return
    except ImportError:
        pass
    import antenv
    from trn_agent_boot.trn_boot import _ntff_profile_via_ctypes

    hook = _ntff_profile_via_ctypes("/opt/axon/libaxon_pjrt.so")
    mod = types.ModuleType("antenv.axon_hooks")
    mod._hook = hook
    mod.get_axon_ntff_profile_hook = lambda: mod._hook
    mod.set_axon_ntff_profile_hook = lambda h: setattr(mod, "_hook", h)
    sys.modules["antenv.axon_hooks"] = mod
    antenv.axon_hooks = mod


def kernel(ufeat, ifeat, Ps, src, dst):
    from concourse.bass_utils import run_bass_kernel_spmd

    ufeat = np.asarray(ufeat, np.float32)
    ifeat = np.asarray(ifeat, np.float32)
    Ps = np.asarray(Ps, np.float32)
    src = np.asarray(src, np.int32)
    dst = np.asarray(dst, np.int32)

    in_maps, metas = _prepare(ufeat, ifeat, Ps, src, dst)
    tiles, group_quarter, NT, NG = _NC_CACHE["layout"]
    if "nc" not in _NC_CACHE:
        _NC_CACHE["nc"] = _build_kernel(None, tiles, group_quarter, NT, NG)
    nc = _NC_CACHE["nc"]
    res = run_bass_kernel_spmd(nc, in_maps, core_ids=list(range(N_CORES)))
    out = np.zeros(E, np.float32)
    for c in range(N_CORES):
        o = res.results[c]["out"]  # [P, NG*TPG]
        flat = o.T.reshape(-1)  # slot id = t*128 + p
        sm = metas[c]
        valid = sm >= 0
        out[sm[valid]] = flat[valid]
    return out
